# revision 1
# baseline (speedup 1.0000x reference)
"""DiT block kernel for Trainium2, 8-core SPMD, no collectives.

Sharding: core c handles batch b = c//2, query-half qh = c%2 (2048 query
tokens). Host permutes each core's x so its query tokens are rows 0..2047;
K/V are computed on-core over all 4096 rows (attention is invariant to key
order). Output gathered on host.

Per-core math (E=384, NH=6, HD=64, FF=1536):
  adaln rows = cond @ [g1|be1|a1|g2|be2|a2] + biases
  scale1 = ln1_w*(1+g1); shift1 = ln1_b*(1+g1)+be1  (same for 2)
  xhat = LN(x) -> PE transpose -> y1T = xhat_T*scale1+shift1   [E,S] bf16
  KT/QT in T-layout (Q scaled by 1/8), V token-layout with ones column
  scoresT[k,q] via row-packed head-pair matmuls (chunk c = heads 2c,2c+1)
  PS = exp(scoresT) bf16 (no max subtraction; scores are small here)
  attnT_unnorm[{d,sum},q] += V_aug^T @ PS  (row 64 = softmax denominators)
  attnT = attnT_unnorm * (1/sums) via K=1 matmul broadcast
  r1 = x + attnT^T @ (wo*alpha1)            (r1 staged in DRAM)
  y2T from LN2(r1); h1T = relu(ff1^T y2T + ff1_b)
  out = r1 + h1T^T @ (ff2*alpha2) + ff2_b*alpha2
"""

import os

os.environ.setdefault("MYCRO_LOCAL_CACHE", "1")

from contextlib import ExitStack

import numpy as np

import concourse.bacc as bacc
import concourse.mybir as mybir
from concourse.masks import make_identity
from concourse.tile import TileContext

F32 = mybir.dt.float32
BF16 = mybir.dt.bfloat16
AF = mybir.ActivationFunctionType
OP = mybir.AluOpType

E = 384
NH = 6
HD = 64
FF = 1536
EPS = 1e-5
NCH = E // 128
NFH = FF // 128
NPAIR = NH // 2


def build_kernel(S_kv=4096, S_q=2048, phases=7):
    """Build the per-core Bass module. phases<7 truncates for debugging and
    dumps the newest intermediate into `out` (rest stays zero)."""
    nc = bacc.Bacc("TRN2", target_bir_lowering=False)

    NKT = S_kv // 128
    NTT = S_kv // 128
    NQT = S_q // 128
    QCH = 512 if S_q % 512 == 0 else S_q
    NQN = S_q // QCH
    NV = S_kv // 512 if S_kv % 512 == 0 else 1
    KCH = S_kv // NV

    xp = nc.dram_tensor("xp", [S_kv, E], F32, kind="ExternalInput")[:, :]
    cond_col = nc.dram_tensor("cond_col", [E, 1], F32, kind="ExternalInput")[:, :]
    adaln_w = nc.dram_tensor("adaln_w", [E, 6 * E], F32, kind="ExternalInput")[:, :]
    adaln_b = nc.dram_tensor("adaln_b", [1, 6 * E], F32, kind="ExternalInput")[:, :]
    ln1w_d = nc.dram_tensor("ln1w", [1, E], F32, kind="ExternalInput")[:, :]
    ln1b_d = nc.dram_tensor("ln1b", [1, E], F32, kind="ExternalInput")[:, :]
    ln2w_d = nc.dram_tensor("ln2w", [1, E], F32, kind="ExternalInput")[:, :]
    ln2b_d = nc.dram_tensor("ln2b", [1, E], F32, kind="ExternalInput")[:, :]
    wq_d = nc.dram_tensor("wq", [E, E], F32, kind="ExternalInput")[:, :]
    wk_d = nc.dram_tensor("wk", [E, E], F32, kind="ExternalInput")[:, :]
    wv_d = nc.dram_tensor("wv", [E, E], F32, kind="ExternalInput")[:, :]
    wo_d = nc.dram_tensor("wo", [E, E], F32, kind="ExternalInput")[:, :]
    ff1_d = nc.dram_tensor("ff1", [E, FF], F32, kind="ExternalInput")[:, :]
    ff1b_d = nc.dram_tensor("ff1b", [1, FF], F32, kind="ExternalInput")[:, :]
    ff2_d = nc.dram_tensor("ff2", [FF, E], F32, kind="ExternalInput")[:, :]
    ff2b_d = nc.dram_tensor("ff2b", [1, E], F32, kind="ExternalInput")[:, :]
    out_d = nc.dram_tensor("out", [S_q, E], F32, kind="ExternalOutput")[:, :]

    ctx = ExitStack()
    with TileContext(nc) as tc, ctx:
        root = ctx.enter_context(tc.tile_pool(name="root", bufs=1))
        dump_pool = ctx.enter_context(tc.tile_pool(name="dmp", bufs=2))

        def dump(ap, row0):
            nr, nco = ap.shape[0], min(ap.shape[-1], E)
            ap = ap[..., 0:nco]
            if len(ap.shape) == 3:
                ap = ap[:, 0, :]
            dt_ = dump_pool.tile([128, E], F32, tag="dt", name="dt")
            nc.vector.tensor_copy(dt_[0:nr, 0:nco], ap)
            nc.sync.dma_start(out_d[row0:row0 + nr, 0:nco], dt_[0:nr, 0:nco])

        ident = root.tile([128, 128], BF16)
        make_identity(nc, ident)
        ones_f = root.tile([1, 128], F32)
        nc.vector.memset(ones_f, 1.0)
        ones_bf = root.tile([1, 128], BF16)
        nc.vector.memset(ones_bf, 1.0)
        eps_t = root.tile([128, 1], F32)
        nc.vector.memset(eps_t, EPS)

        ln1w = root.tile([1, E], F32); nc.sync.dma_start(ln1w, ln1w_d)
        ln1b = root.tile([1, E], F32); nc.sync.dma_start(ln1b, ln1b_d)
        ln2w = root.tile([1, E], F32); nc.sync.dma_start(ln2w, ln2w_d)
        ln2b = root.tile([1, E], F32); nc.sync.dma_start(ln2b, ln2b_d)
        adab = root.tile([1, 6 * E], F32); nc.sync.dma_start(adab, adaln_b)
        ff2b_r = root.tile([1, E], F32); nc.sync.dma_start(ff2b_r, ff2b_d)
        cond_sb = root.tile([128, NCH, 1], F32)
        nc.sync.dma_start(cond_sb, cond_col.rearrange("(c p) o -> p c o", p=128))
        cond_bf = root.tile([128, NCH, 1], BF16)
        nc.vector.tensor_copy(cond_bf, cond_sb)

        # ---------- phase 0: AdaLN projections ----------
        adaln_rows = root.tile([1, 6, E], F32)  # g1 be1 a1 g2 be2 a2
        with tc.tile_pool(name="ph0", bufs=1) as p0, \
             tc.tile_pool(name="ph0p", bufs=2, space="PSUM") as p0p:
            aw = p0.tile([128, NCH, 6 * E], F32)
            nc.sync.dma_start(aw, adaln_w.rearrange("(c p) n -> p c n", p=128))
            aw_bf = p0.tile([128, NCH, 6 * E], BF16)
            nc.vector.tensor_copy(aw_bf, aw)
            for j in range(6):
                ps = p0p.tile([1, E], F32, tag="adps", name="adps")
                for k in range(NCH):
                    nc.tensor.matmul(ps, cond_bf[:, k, :],
                                     aw_bf[:, k, j * E:(j + 1) * E],
                                     start=(k == 0), stop=(k == NCH - 1))
                nc.vector.tensor_tensor(adaln_rows[:, j, :], ps,
                                        adab[:, j * E:(j + 1) * E], OP.add)

        g1p = root.tile([1, E], F32)
        nc.vector.tensor_scalar(g1p, adaln_rows[:, 0, :], 1.0, None, OP.add)
        g2p = root.tile([1, E], F32)
        nc.vector.tensor_scalar(g2p, adaln_rows[:, 3, :], 1.0, None, OP.add)
        scale1_r = root.tile([1, E], F32)
        nc.vector.tensor_tensor(scale1_r, g1p, ln1w, OP.mult)
        scale2_r = root.tile([1, E], F32)
        nc.vector.tensor_tensor(scale2_r, g2p, ln2w, OP.mult)
        shift1_r = root.tile([1, E], F32)
        nc.vector.tensor_tensor(shift1_r, g1p, ln1b, OP.mult)
        nc.vector.tensor_tensor(shift1_r, shift1_r, adaln_rows[:, 1, :], OP.add)
        shift2_r = root.tile([1, E], F32)
        nc.vector.tensor_tensor(shift2_r, g2p, ln2b, OP.mult)
        nc.vector.tensor_tensor(shift2_r, shift2_r, adaln_rows[:, 4, :], OP.add)

        scale1_c = root.tile([128, NCH], F32)
        shift1_c = root.tile([128, NCH], F32)
        scale2_c = root.tile([128, NCH], F32)
        shift2_c = root.tile([128, NCH], F32)
        for c in range(NCH):
            s = slice(c * 128, (c + 1) * 128)
            nc.sync.dma_start(scale1_c[:, c:c + 1], scale1_r[:, s])
            nc.sync.dma_start(shift1_c[:, c:c + 1], shift1_r[:, s])
            nc.sync.dma_start(scale2_c[:, c:c + 1], scale2_r[:, s])
            nc.sync.dma_start(shift2_c[:, c:c + 1], shift2_r[:, s])
        ff1b_c = root.tile([128, NFH], F32)
        for c in range(NFH):
            nc.sync.dma_start(ff1b_c[:, c:c + 1], ff1b_d[:, c * 128:(c + 1) * 128])

        alpha1_b = root.tile([128, E], F32)
        alpha2_b = root.tile([128, E], F32)
        with tc.tile_pool(name="abp", bufs=2, space="PSUM") as abp:
            psa = abp.tile([128, E], F32)
            nc.tensor.matmul(psa, ones_f, adaln_rows[:, 2, :], start=True, stop=True)
            nc.vector.tensor_copy(alpha1_b, psa)
            psb_ = abp.tile([128, E], F32)
            nc.tensor.matmul(psb_, ones_f, adaln_rows[:, 5, :], start=True, stop=True)
            nc.vector.tensor_copy(alpha2_b, psb_)

        fb_bf = root.tile([1, E], BF16)
        nc.vector.tensor_tensor(fb_bf, ff2b_r, adaln_rows[:, 5, :], OP.mult)

        wo_bf = root.tile([64, NH, E], BF16)

        # ff weights loaded/cast early so they overlap LN1/QKV/attention
        pffw = ctx.enter_context(tc.tile_pool(name="pffw", bufs=1))
        ff1_bf = pffw.tile([128, NCH, FF], BF16)
        ff2_bf = pffw.tile([128, NFH, E], BF16)
        with tc.tile_pool(name="fstg", bufs=1) as fst:
            f1 = fst.tile([128, NCH, FF], F32)
            nc.sync.dma_start(f1, ff1_d.rearrange("(c p) n -> p c n", p=128))
            nc.vector.tensor_copy(ff1_bf, f1)
            f2 = fst.tile([128, NFH, E], F32)
            nc.sync.dma_start(f2, ff2_d.rearrange("(c p) n -> p c n", p=128))
            for k in range(NFH):
                nc.vector.tensor_tensor(ff2_bf[:, k, :], f2[:, k, :],
                                        alpha2_b, OP.mult)

        with ExitStack() as kv:
            pkv = kv.enter_context(tc.tile_pool(name="pkv", bufs=1))
            KT = [pkv.tile([128, S_kv], BF16, tag=f"KT{c}", name=f"KT{c}")
                  for c in range(NCH)]
            QT = [pkv.tile([128, S_q], BF16, tag=f"QT{c}", name=f"QT{c}")
                  for c in range(NCH)]
            V_sb = pkv.tile([128, NKT, NH, HD + 1], BF16)
            nc.vector.memset(V_sb[:, :, :, HD:HD + 1], 1.0)
            wq_bf = pkv.tile([128, NCH, E], BF16)
            wk_bf = pkv.tile([128, NCH, E], BF16)
            wv_bf = pkv.tile([128, NCH, E], BF16)

            with tc.tile_pool(name="wstg", bufs=1) as wst:
                for w_d, w_bf in ((wq_d, wq_bf), (wk_d, wk_bf), (wv_d, wv_bf)):
                    wf = wst.tile([128, NCH, E], F32, tag="wstage", name="wstage")
                    nc.sync.dma_start(wf, w_d.rearrange("(c p) n -> p c n", p=128))
                    nc.vector.tensor_copy(w_bf, wf)
                wof = wst.tile([64, NH, E], F32, tag="wostage", name="wostage")
                nc.sync.dma_start(wof, wo_d.rearrange("(h d) n -> d h n", h=NH))
                for h in range(NH):
                    nc.vector.tensor_tensor(wo_bf[:, h, :], wof[:, h, :],
                                            alpha1_b[0:64, :], OP.mult)

            # ---------- phases 1-2: LN1 + transpose + modulate; QKV ----------
            with ExitStack() as y1s:
                py1 = y1s.enter_context(tc.tile_pool(name="y1", bufs=1))
                y1T = [py1.tile([128, S_kv], BF16, tag=f"y1T{c}", name=f"y1T{c}")
                       for c in range(NCH)]
                with tc.tile_pool(name="ln1", bufs=4) as pln, \
                     tc.tile_pool(name="ln1p", bufs=3, space="PSUM") as plnp:
                    for i in range(NTT):
                        xt = pln.tile([128, E], F32, tag="xt", name="xt")
                        nc.sync.dma_start(xt, xp[i * 128:(i + 1) * 128, :])
                        st = pln.tile([128, 6], F32, tag="st", name="st")
                        nc.vector.bn_stats(st, xt)
                        mv = pln.tile([128, 2], F32, tag="mv", name="mv")
                        nc.vector.bn_aggr(mv, st)
                        sd = pln.tile([128, 1], F32, tag="sd", name="sd")
                        nc.scalar.activation(sd, mv[:, 1:2], AF.Sqrt, bias=eps_t)
                        rstd = pln.tile([128, 1], F32, tag="rstd", name="rstd")
                        nc.vector.reciprocal(rstd, sd)
                        nmr = pln.tile([128, 1], F32, tag="nmr", name="nmr")
                        nc.vector.tensor_scalar(nmr, mv[:, 0:1], rstd, -1.0,
                                                OP.mult, OP.mult)
                        xh = pln.tile([128, E], BF16, tag="xh", name="xh")
                        nc.vector.tensor_scalar(xh, xt, rstd, nmr, OP.mult, OP.add)
                        for c in range(NCH):
                            pst = plnp.tile([128, 128], BF16, tag="pst", name="pst")
                            nc.tensor.transpose(pst, xh[:, c * 128:(c + 1) * 128],
                                                ident)
                            nc.vector.tensor_scalar(
                                y1T[c][:, i * 128:(i + 1) * 128], pst,
                                scale1_c[:, c:c + 1], shift1_c[:, c:c + 1],
                                OP.mult, OP.add)

                if phases < 2:
                    dump(y1T[0][:, 0:min(E, S_kv)], 0)
                else:
                    with tc.tile_pool(name="qkp", bufs=4, space="PSUM") as qkp, \
                         tc.tile_pool(name="vp", bufs=2, space="PSUM") as vpp:
                        for m in range(NKT):
                            ps = vpp.tile([128, E], F32, tag="v", name="v")
                            for k in range(NCH):
                                nc.tensor.matmul(ps,
                                                 y1T[k][:, m * 128:(m + 1) * 128],
                                                 wv_bf[:, k, :],
                                                 start=(k == 0), stop=(k == NCH - 1))
                            nc.vector.tensor_copy(
                                V_sb[:, m, :, 0:HD],
                                ps.rearrange("p (h d) -> p h d", h=NH))
                        for c in range(NCH):
                            for n in range(NV):
                                ps = qkp.tile([128, KCH], F32, tag="qk", name="qk")
                                for k in range(NCH):
                                    nc.tensor.matmul(
                                        ps, wk_bf[:, k, c * 128:(c + 1) * 128],
                                        y1T[k][:, n * KCH:(n + 1) * KCH],
                                        start=(k == 0), stop=(k == NCH - 1))
                                nc.vector.tensor_copy(
                                    KT[c][:, n * KCH:(n + 1) * KCH], ps)
                            for n in range(NQN):
                                ps = qkp.tile([128, QCH], F32, tag="qk", name="qk")
                                for k in range(NCH):
                                    nc.tensor.matmul(
                                        ps, wq_bf[:, k, c * 128:(c + 1) * 128],
                                        y1T[k][:, n * QCH:(n + 1) * QCH],
                                        start=(k == 0), stop=(k == NCH - 1))
                                nc.vector.tensor_scalar(
                                    QT[c][:, n * QCH:(n + 1) * QCH], ps,
                                    0.125, None, OP.mult)

            # ---------- phase 3+: attention with per-qn fused downstream ----
            if phases == 2:
                dump(KT[0][:, 0:min(E, S_kv)], 0)
                dump(QT[0][:, 0:min(E, S_q)], 128)
            if phases >= 3:
                HB = 512
                TQ = QCH // 128
                with tc.tile_pool(name="ps_sb", bufs=2) as psb, \
                     tc.tile_pool(name="nrm", bufs=1) as nrm, \
                     tc.tile_pool(name="attq", bufs=2) as attq_p, \
                     tc.tile_pool(name="dsb", bufs=2) as dsb, \
                     tc.tile_pool(name="dwk", bufs=2) as dwk, \
                     tc.tile_pool(name="sco", bufs=2, space="PSUM") as sco, \
                     tc.tile_pool(name="acc", bufs=1, space="PSUM") as acc, \
                     tc.tile_pool(name="dsp", bufs=2, space="PSUM") as dsp:
                    for qn in range(NQN):
                        qs = slice(qn * QCH, (qn + 1) * QCH)
                        atq = attq_p.tile([64, NH, QCH], BF16, tag="atq",
                                          name="atq")
                        for p in range(NPAIR):
                            h0, h1 = 2 * p, 2 * p + 1
                            pa = acc.tile([65, 2 * HB], F32, tag="pa", name="pa")
                            for kt in range(NKT):
                                ks = slice(kt * 128, (kt + 1) * 128)
                                ss = sco.tile([128, 2 * HB], F32, tag="ss",
                                              name="ss")
                                nc.tensor.matmul(ss[:, 0:QCH], KT[p][0:64, ks],
                                                 QT[p][0:64, qs],
                                                 start=True, stop=True,
                                                 tile_position=(0, 0))
                                nc.tensor.matmul(ss[:, HB:HB + QCH],
                                                 KT[p][64:128, ks],
                                                 QT[p][64:128, qs],
                                                 start=True, stop=True,
                                                 tile_position=(64, 0))
                                ex = psb.tile([128, 2 * HB], BF16, tag="ex",
                                              name="ex")
                                if QCH == HB:
                                    nc.scalar.activation(ex, ss, AF.Exp)
                                else:
                                    nc.scalar.activation(ex[:, 0:QCH],
                                                         ss[:, 0:QCH], AF.Exp)
                                    nc.scalar.activation(ex[:, HB:HB + QCH],
                                                         ss[:, HB:HB + QCH],
                                                         AF.Exp)
                                nc.tensor.matmul(pa[:, 0:QCH], V_sb[:, kt, h0, :],
                                                 ex[:, 0:QCH],
                                                 start=(kt == 0),
                                                 stop=(kt == NKT - 1))
                                nc.tensor.matmul(pa[:, HB:HB + QCH],
                                                 V_sb[:, kt, h1, :],
                                                 ex[:, HB:HB + QCH],
                                                 start=(kt == 0),
                                                 stop=(kt == NKT - 1))
                            ta = nrm.tile([65, 2 * HB], F32, tag="ta", name="ta")
                            nc.vector.tensor_copy(ta[:, 0:QCH], pa[:, 0:QCH])
                            nc.vector.tensor_copy(ta[:, HB:HB + QCH],
                                                  pa[:, HB:HB + QCH])
                            sums = nrm.tile([1, 2 * HB], F32, tag="sums",
                                            name="sums")
                            nc.sync.dma_start(sums[:, 0:QCH], ta[64:65, 0:QCH])
                            nc.sync.dma_start(sums[:, HB:HB + QCH],
                                              ta[64:65, HB:HB + QCH])
                            rrow = nrm.tile([1, 2 * HB], F32, tag="rrow",
                                            name="rrow")
                            nc.vector.reciprocal(rrow[:, 0:QCH], sums[:, 0:QCH])
                            nc.vector.reciprocal(rrow[:, HB:HB + QCH],
                                                 sums[:, HB:HB + QCH])
                            rbc = sco.tile([64, 2 * HB], F32, tag="ss", name="rbc")
                            nc.tensor.matmul(rbc[:, 0:QCH], ones_f[:, 0:64],
                                             rrow[:, 0:QCH],
                                             start=True, stop=True)
                            nc.tensor.matmul(rbc[:, HB:HB + QCH], ones_f[:, 0:64],
                                             rrow[:, HB:HB + QCH],
                                             start=True, stop=True)
                            nc.vector.tensor_tensor(atq[:, h0, :],
                                                    ta[0:64, 0:QCH],
                                                    rbc[:, 0:QCH], OP.mult)
                            nc.vector.tensor_tensor(atq[:, h1, :],
                                                    ta[0:64, HB:HB + QCH],
                                                    rbc[:, HB:HB + QCH], OP.mult)

                        if phases == 3:
                            if qn == 0:
                                dump(atq[:, 0, 0:min(E, QCH)], 0)
                                dump(atq[:, 1, 0:min(E, QCH)], 64)
                            continue

                        # ---- fused downstream for this q-chunk ----
                        r1q = dsb.tile([128, TQ, E], F32, tag="r1q", name="r1q")
                        y2q = dsb.tile([128, NCH, QCH], BF16, tag="y2q",
                                       name="y2q")
                        for t in range(TQ):
                            trow = qn * QCH + t * 128
                            ps = dsp.tile([128, E], F32, tag="dsp", name="wops")
                            for h in range(NH):
                                nc.tensor.matmul(
                                    ps, atq[:, h, t * 128:(t + 1) * 128],
                                    wo_bf[:, h, :],
                                    start=(h == 0), stop=(h == NH - 1))
                            xq = dwk.tile([128, E], F32, tag="xq", name="xq")
                            nc.sync.dma_start(xq, xp[trow:trow + 128, :])
                            nc.vector.tensor_tensor(r1q[:, t, :], ps, xq, OP.add)
                            # LN2 on this tile
                            st = dwk.tile([128, 6], F32, tag="st2", name="st2")
                            nc.vector.bn_stats(st, r1q[:, t, :])
                            mv = dwk.tile([128, 2], F32, tag="mv2", name="mv2")
                            nc.vector.bn_aggr(mv, st)
                            sd = dwk.tile([128, 1], F32, tag="sd2", name="sd2")
                            nc.scalar.activation(sd, mv[:, 1:2], AF.Sqrt,
                                                 bias=eps_t)
                            rstd = dwk.tile([128, 1], F32, tag="rstd2",
                                            name="rstd2")
                            nc.vector.reciprocal(rstd, sd)
                            nmr = dwk.tile([128, 1], F32, tag="nmr2", name="nmr2")
                            nc.vector.tensor_scalar(nmr, mv[:, 0:1], rstd, -1.0,
                                                    OP.mult, OP.mult)
                            xh = dwk.tile([128, E], BF16, tag="xh2", name="xh2")
                            nc.vector.tensor_scalar(xh, r1q[:, t, :], rstd, nmr,
                                                    OP.mult, OP.add)
                            for c in range(NCH):
                                pst = dsp.tile([128, 128], BF16, tag="dsp",
                                               name="pst2")
                                nc.tensor.transpose(
                                    pst, xh[:, c * 128:(c + 1) * 128], ident)
                                nc.vector.tensor_scalar(
                                    y2q[:, c, t * 128:(t + 1) * 128], pst,
                                    scale2_c[:, c:c + 1], shift2_c[:, c:c + 1],
                                    OP.mult, OP.add)

                        h1q = dsb.tile([128, NFH, QCH], BF16, tag="h1q",
                                       name="h1q")
                        for m in range(NFH):
                            ps = dsp.tile([128, QCH], F32, tag="dsp", name="f1ps")
                            for k in range(NCH):
                                nc.tensor.matmul(
                                    ps, ff1_bf[:, k, m * 128:(m + 1) * 128],
                                    y2q[:, k, :],
                                    start=(k == 0), stop=(k == NCH - 1))
                            nc.vector.tensor_scalar(
                                h1q[:, m, :], ps,
                                ff1b_c[:, m:m + 1], 0.0, OP.add, OP.max)

                        for t in range(TQ):
                            trow = qn * QCH + t * 128
                            ps = dsp.tile([128, E], F32, tag="dsp", name="f2ps")
                            for k in range(NFH):
                                nc.tensor.matmul(
                                    ps, h1q[:, k, t * 128:(t + 1) * 128],
                                    ff2_bf[:, k, :],
                                    start=(k == 0), stop=False)
                            nc.tensor.matmul(ps, ones_bf, fb_bf,
                                             start=False, stop=True)
                            ot = dwk.tile([128, E], F32, tag="ot", name="ot")
                            nc.vector.tensor_tensor(ot, ps, r1q[:, t, :], OP.add)
                            nc.sync.dma_start(out_d[trow:trow + 128, :], ot)

    nc.finalize()
    return nc


_NC_CACHE = {}


def _get_nc(S_kv, S_q):
    key = (S_kv, S_q)
    if key not in _NC_CACHE:
        _NC_CACHE[key] = build_kernel(S_kv, S_q)
    return _NC_CACHE[key]


def make_in_maps(inputs, n_cores=8, S=4096):
    """Shard FULL inputs into per-core input maps."""
    x = np.asarray(inputs["x"], np.float32)
    cond = np.asarray(inputs["cond"], np.float32)
    Sq = S // 2
    adaln_w = np.concatenate(
        [np.asarray(inputs[k], np.float32)
         for k in ("g1_w", "be1_w", "a1_w", "g2_w", "be2_w", "a2_w")], axis=1)
    adaln_b = np.concatenate(
        [np.asarray(inputs[k], np.float32)
         for k in ("g1_b", "be1_b", "a1_b", "g2_b", "be2_b", "a2_b")])[None, :]
    shared = {
        "adaln_w": np.ascontiguousarray(adaln_w),
        "adaln_b": np.ascontiguousarray(adaln_b),
        "ln1w": np.asarray(inputs["ln1_w"], np.float32)[None, :],
        "ln1b": np.asarray(inputs["ln1_b"], np.float32)[None, :],
        "ln2w": np.asarray(inputs["ln2_w"], np.float32)[None, :],
        "ln2b": np.asarray(inputs["ln2_b"], np.float32)[None, :],
        "wq": np.asarray(inputs["wq"], np.float32),
        "wk": np.asarray(inputs["wk"], np.float32),
        "wv": np.asarray(inputs["wv"], np.float32),
        "wo": np.asarray(inputs["wo"], np.float32),
        "ff1": np.asarray(inputs["ff1_w"], np.float32),
        "ff1b": np.asarray(inputs["ff1_b"], np.float32)[None, :],
        "ff2": np.asarray(inputs["ff2_w"], np.float32),
        "ff2b": np.asarray(inputs["ff2_b"], np.float32)[None, :],
    }
    in_maps = []
    for c in range(n_cores):
        b, qh = c // 2, c % 2
        xb = x[b]
        xpm = np.concatenate([xb[qh * Sq:(qh + 1) * Sq],
                              xb[(1 - qh) * Sq:(2 - qh) * Sq]], axis=0)
        m = dict(shared)
        m["xp"] = np.ascontiguousarray(xpm)
        m["cond_col"] = np.ascontiguousarray(cond[b].reshape(E, 1))
        in_maps.append(m)
    return in_maps


def kernel(**inputs):
    from concourse.bass_utils import run_bass_kernel_spmd

    x = np.asarray(inputs["x"], np.float32)
    B, S, _ = x.shape
    Sq = S // 2
    nc = _get_nc(S, Sq)
    in_maps = make_in_maps(inputs, n_cores=8, S=S)
    res = run_bass_kernel_spmd(nc, in_maps, core_ids=list(range(8)))
    out = np.empty((B, S, E), np.float32)
    for c in range(8):
        b, qh = c // 2, c % 2
        out[b, qh * Sq:(qh + 1) * Sq] = res.results[c]["out"]
    return out



# revision 26
# speedup vs baseline: 42.7250x; 42.7250x over previous
"""DiT block kernel for Trainium2, 8-core SPMD, no collectives.

Sharding: core c handles batch b = c//2, query-half qh = c%2 (2048 query
tokens). Host permutes each core's x so its query tokens are rows 0..2047;
K/V are computed on-core over all 4096 rows (attention is invariant to key
order). Output gathered on host.

Per-core math (E=384, NH=6, HD=64, FF=1536):
  adaln rows = cond @ [g1|be1|a1|g2|be2|a2] + biases
  scale1 = ln1_w*(1+g1); shift1 = ln1_b*(1+g1)+be1  (same for 2)
  LN1: bn_stats; rstd = exp(-0.5*ln(var+eps)) (keeps ACT on one table set)
  xh bf16 -> DMA-xbar transpose -> y1T [E,S]; modulate in T-layout
  KT/QT via matmul + ScalarE copy (Q unscaled; 1/8 folded into exp scale)
  V token-layout with ones column for softmax denominators
  scores: per kt, pair-packed row matmuls -> ss [128,1024] PSUM
  ex = exp(ss/8) one ACT op FD=1024 -> bf16 SBUF
  pa[{d,sum} x 2 heads, q] += V_aug^T @ ex  (PSUM accum over kt)
  attnT = pa * (1/sums) via K=1 broadcast matmul + DVE mult
  wo pair-packed (K=128) matmuls; r1 = x + attn; LN2 same rstd trick
  y2T via DMA transpose; h1 = relu(ff1^T y2T + b); out = r1 + ff2^T h1*alpha2
"""

import os

os.environ.setdefault("MYCRO_LOCAL_CACHE", "1")

from contextlib import ExitStack

import numpy as np

import concourse.bacc as bacc
import concourse.mybir as mybir
from concourse.tile import TileContext

F32 = mybir.dt.float32
BF16 = mybir.dt.bfloat16
AF = mybir.ActivationFunctionType
OP = mybir.AluOpType

E = 384
NH = 6
HD = 64
FF = 1536
EPS = 1e-5
NCH = E // 128
NFH = FF // 128
NPAIR = NH // 2


def build_kernel(S_kv=4096, S_q=2048, reps=1):
    nc = bacc.Bacc("TRN2", target_bir_lowering=False)

    NKT = S_kv // 128
    NTT = S_kv // 128
    QCH = 512 if S_q % 512 == 0 else S_q
    NQN = S_q // QCH
    TQ = QCH // 128
    KCH = 512 if S_kv % 512 == 0 else S_kv
    NKC = S_kv // KCH

    xp = nc.dram_tensor("xp", [S_kv, E], F32, kind="ExternalInput")[:, :]
    cond_col = nc.dram_tensor("cond_col", [E, 1], F32, kind="ExternalInput")[:, :]
    adaln_w = nc.dram_tensor("adaln_w", [E, 6 * E], F32, kind="ExternalInput")[:, :]
    adaln_b = nc.dram_tensor("adaln_b", [1, 6 * E], F32, kind="ExternalInput")[:, :]
    ln1w_d = nc.dram_tensor("ln1w", [1, E], F32, kind="ExternalInput")[:, :]
    ln1b_d = nc.dram_tensor("ln1b", [1, E], F32, kind="ExternalInput")[:, :]
    ln2w_d = nc.dram_tensor("ln2w", [1, E], F32, kind="ExternalInput")[:, :]
    ln2b_d = nc.dram_tensor("ln2b", [1, E], F32, kind="ExternalInput")[:, :]
    wq_d = nc.dram_tensor("wq", [E, E], F32, kind="ExternalInput")[:, :]
    wk_d = nc.dram_tensor("wk", [E, E], F32, kind="ExternalInput")[:, :]
    wv_d = nc.dram_tensor("wv", [E, E], F32, kind="ExternalInput")[:, :]
    wo_d = nc.dram_tensor("wo", [E, E], F32, kind="ExternalInput")[:, :]
    ff1_d = nc.dram_tensor("ff1", [E, FF], F32, kind="ExternalInput")[:, :]
    ff1b_d = nc.dram_tensor("ff1b", [1, FF], F32, kind="ExternalInput")[:, :]
    ff2_d = nc.dram_tensor("ff2", [FF, E], F32, kind="ExternalInput")[:, :]
    ff2b_d = nc.dram_tensor("ff2b", [1, E], F32, kind="ExternalInput")[:, :]
    out_d = nc.dram_tensor("out", [S_q, E], F32, kind="ExternalOutput")[:, :]

    ctx = ExitStack()
    with TileContext(nc) as tc, ctx:
        root = ctx.enter_context(tc.tile_pool(name="root", bufs=1))

        # Pin the ACT table set that has ln+exp+copy so the auto-placement
        # pass never needs to swap sets (it thrashes between the ln-only and
        # exp-only sets otherwise).
        from concourse.hw_specs import get_activation_tables
        _set_id = list(get_activation_tables(nc.m.arch)).index(
            "natural_log_exp_and_others")
        _actload = mybir.InstLoadActFuncSet(name="pin_actload", ins=[], outs=[])
        _actload.engine = mybir.EngineType.Activation
        _actload.act_func_set_id = _set_id
        nc.scalar.add_instruction(_actload)

        ones_f = root.tile([1, 128], F32)
        nc.vector.memset(ones_f, 1.0)
        ones_bf = root.tile([1, 128], BF16)
        nc.vector.memset(ones_bf, 1.0)
        eps_t = root.tile([128, 1], F32)
        nc.vector.memset(eps_t, EPS)

        ln1w = root.tile([1, E], F32); nc.sync.dma_start(ln1w, ln1w_d)
        ln1b = root.tile([1, E], F32); nc.sync.dma_start(ln1b, ln1b_d)
        ln2w = root.tile([1, E], F32); nc.sync.dma_start(ln2w, ln2w_d)
        ln2b = root.tile([1, E], F32); nc.sync.dma_start(ln2b, ln2b_d)
        adab = root.tile([1, 6 * E], F32); nc.sync.dma_start(adab, adaln_b)
        ff2b_r = root.tile([1, E], F32); nc.sync.dma_start(ff2b_r, ff2b_d)
        cond_bf = root.tile([128, NCH, 1], BF16)
        nc.gpsimd.dma_start(cond_bf, cond_col.rearrange("(c p) o -> p c o", p=128))

        # bf16 weights, cast during SWDGE DMA (Pool engine, otherwise idle).
        # Load order matters: adaln + qkv weights gate early compute; wo/ff
        # are not needed until the first q-chunk's downstream.
        wq_bf = root.tile([128, NCH, E], BF16)
        nc.gpsimd.dma_start(wq_bf, wq_d.rearrange("(c p) n -> p c n", p=128))
        wk_bf = root.tile([128, NCH, E], BF16)
        nc.gpsimd.dma_start(wk_bf, wk_d.rearrange("(c p) n -> p c n", p=128))
        wv_bf = root.tile([128, NCH, E], BF16)
        nc.gpsimd.dma_start(wv_bf, wv_d.rearrange("(c p) n -> p c n", p=128))

        # ---------- phase 0: AdaLN projections ----------
        adaln_rows = root.tile([1, 6, E], F32)  # g1 be1 a1 g2 be2 a2
        with tc.tile_pool(name="ph0", bufs=1) as p0, \
             tc.tile_pool(name="ph0p", bufs=2, space="PSUM") as p0p:
            aw_bf = p0.tile([128, NCH, 6 * E], BF16)
            nc.gpsimd.dma_start(aw_bf, adaln_w.rearrange("(c p) n -> p c n", p=128))
            for j in range(6):
                ps = p0p.tile([1, E], F32, tag="adps", name="adps")
                for k in range(NCH):
                    nc.tensor.matmul(ps, cond_bf[:, k, :],
                                     aw_bf[:, k, j * E:(j + 1) * E],
                                     start=(k == 0), stop=(k == NCH - 1))
                nc.vector.tensor_tensor(adaln_rows[:, j, :], ps,
                                        adab[:, j * E:(j + 1) * E], OP.add)

        ff1_bf = root.tile([128, NCH, FF], BF16)
        nc.gpsimd.dma_start(ff1_bf, ff1_d.rearrange("(c p) n -> p c n", p=128))
        wstg = tc.tile_pool(name="wstg", bufs=1)
        wpool = wstg.__enter__()
        # wo in pair layout: partition = (h%2)*64+d, mid index = pair
        wof_bf = wpool.tile([128, NPAIR, E], BF16)
        nc.gpsimd.dma_start(
            wof_bf, wo_d.rearrange("(pr two d) n -> (two d) pr n", two=2, d=HD))
        ff2f_bf = wpool.tile([128, NFH, E], BF16)
        nc.gpsimd.dma_start(ff2f_bf, ff2_d.rearrange("(c p) n -> p c n", p=128))

        g1p = root.tile([1, E], F32)
        nc.vector.tensor_scalar(g1p, adaln_rows[:, 0, :], 1.0, None, OP.add)
        g2p = root.tile([1, E], F32)
        nc.vector.tensor_scalar(g2p, adaln_rows[:, 3, :], 1.0, None, OP.add)
        scale1_r = root.tile([1, E], F32)
        nc.vector.tensor_tensor(scale1_r, g1p, ln1w, OP.mult)
        scale2_r = root.tile([1, E], F32)
        nc.vector.tensor_tensor(scale2_r, g2p, ln2w, OP.mult)
        shift1_r = root.tile([1, E], F32)
        nc.vector.tensor_tensor(shift1_r, g1p, ln1b, OP.mult)
        nc.vector.tensor_tensor(shift1_r, shift1_r, adaln_rows[:, 1, :], OP.add)
        shift2_r = root.tile([1, E], F32)
        nc.vector.tensor_tensor(shift2_r, g2p, ln2b, OP.mult)
        nc.vector.tensor_tensor(shift2_r, shift2_r, adaln_rows[:, 4, :], OP.add)

        # row -> column transposes via K=1/N=1 matmuls (PE idles in phase 0;
        # small HWDGE DMAs here would clog the ramp to phase 1)
        def col_mms(ps, j0, row, nch=NCH):
            for c in range(nch):
                nc.tensor.matmul(ps[:, j0 + c:j0 + c + 1],
                                 row[:, c * 128:(c + 1) * 128],
                                 ones_f[:, 0:1], start=True, stop=True)

        scl_cols = root.tile([128, 4, NCH], F32)
        ff1b_r = root.tile([1, FF], F32)
        nc.sync.dma_start(ff1b_r, ff1b_d)
        ff1b_c = root.tile([128, NFH], F32)
        with tc.tile_pool(name="colp", bufs=2, space="PSUM") as colp:
            cps = colp.tile([128, 4 * NCH], F32, tag="cps", name="cps")
            col_mms(cps, 0, scale1_r)
            col_mms(cps, NCH, shift1_r)
            col_mms(cps, 2 * NCH, scale2_r)
            col_mms(cps, 3 * NCH, shift2_r)
            nc.vector.tensor_copy(scl_cols.rearrange("p a c -> p (a c)"), cps)
            fps = colp.tile([128, NFH], F32, tag="fps2", name="fps2")
            col_mms(fps, 0, ff1b_r, nch=NFH)
            nc.vector.tensor_copy(ff1b_c, fps)
        scale1_c = scl_cols[:, 0, :]
        shift1_c = scl_cols[:, 1, :]
        scale2_c = scl_cols[:, 2, :]
        shift2_c = scl_cols[:, 3, :]

        # Fold the AdaLN modulation into the projection weights:
        #   y1 = xhat*scale1 + shift1  =>  y1 @ W = xhat @ (rows of W scaled
        #   by scale1) + (shift1 @ W).  Biases land per out-channel: columns
        #   for KT/QT (applied in the PSUM->SBUF copy), a row for V (extra
        #   rank-1 matmul), and an addend to ff1's bias column.
        shift1_cb = root.tile([128, NCH], BF16)
        nc.vector.tensor_copy(shift1_cb, shift1_c)
        shift2_cb = root.tile([128, NCH], BF16)
        nc.vector.tensor_copy(shift2_cb, shift2_c)
        kb_col = root.tile([128, NCH], F32)
        qb_col = root.tile([128, NCH], F32)
        vb_bf = root.tile([1, E], BF16)
        with tc.tile_pool(name="fold", bufs=2) as fp, \
             tc.tile_pool(name="foldp", bufs=2, space="PSUM") as fpp:
            for w_bf, dstcol in ((wk_bf, kb_col), (wq_bf, qb_col)):
                ps = fpp.tile([1, E], F32, tag="fps", name="fps")
                for k in range(NCH):
                    nc.tensor.matmul(ps, shift1_cb[:, k:k + 1], w_bf[:, k, :],
                                     start=(k == 0), stop=(k == NCH - 1))
                row = fp.tile([1, E], F32, tag="frow", name="frow")
                nc.vector.tensor_copy(row, ps)
                cp = fpp.tile([128, NCH], F32, tag="fcol", name="fcol")
                col_mms(cp, 0, row)
                nc.vector.tensor_copy(dstcol, cp)
            ps = fpp.tile([1, E], F32, tag="fps", name="fps")
            for k in range(NCH):
                nc.tensor.matmul(ps, shift1_cb[:, k:k + 1], wv_bf[:, k, :],
                                 start=(k == 0), stop=(k == NCH - 1))
            nc.vector.tensor_copy(vb_bf, ps)
            fb1row = fp.tile([1, FF], F32, tag="fb1row", name="fb1row")
            for n in range(FF // 512):
                ps = fpp.tile([1, 512], F32, tag="f512", name="f512")
                for k in range(NCH):
                    nc.tensor.matmul(ps, shift2_cb[:, k:k + 1],
                                     ff1_bf[:, k, n * 512:(n + 1) * 512],
                                     start=(k == 0), stop=(k == NCH - 1))
                nc.vector.tensor_copy(fb1row[:, n * 512:(n + 1) * 512], ps)
            fcp = fpp.tile([128, NFH], F32, tag="fb1c", name="fb1c")
            col_mms(fcp, 0, fb1row, nch=NFH)
            fb1_col = fp.tile([128, NFH], F32, tag="fb1cs", name="fb1cs")
            nc.vector.tensor_copy(fb1_col, fcp)
            nc.vector.tensor_tensor(ff1b_c, ff1b_c, fb1_col, OP.add)
        # scale weight rows in place (WAR-ordered after the bias matmuls)
        for k in range(NCH):
            nc.vector.tensor_scalar(wq_bf[:, k, :], wq_bf[:, k, :],
                                    scale1_c[:, k:k + 1], None, OP.mult)
            nc.vector.tensor_scalar(wk_bf[:, k, :], wk_bf[:, k, :],
                                    scale1_c[:, k:k + 1], None, OP.mult)
            nc.vector.tensor_scalar(wv_bf[:, k, :], wv_bf[:, k, :],
                                    scale1_c[:, k:k + 1], None, OP.mult)
            nc.vector.tensor_scalar(ff1_bf[:, k, :], ff1_bf[:, k, :],
                                    scale2_c[:, k:k + 1], None, OP.mult)

        alpha1_bf = root.tile([128, E], BF16)
        alpha2_bf = root.tile([128, E], BF16)
        with tc.tile_pool(name="abp", bufs=2, space="PSUM") as abp:
            psa = abp.tile([128, E], F32)
            nc.tensor.matmul(psa, ones_f, adaln_rows[:, 2, :], start=True, stop=True)
            nc.vector.tensor_copy(alpha1_bf, psa)
            psb_ = abp.tile([128, E], F32)
            nc.tensor.matmul(psb_, ones_f, adaln_rows[:, 5, :], start=True, stop=True)
            nc.vector.tensor_copy(alpha2_bf, psb_)

        fb_bf = root.tile([1, E], BF16)
        nc.vector.tensor_tensor(fb_bf, ff2b_r, adaln_rows[:, 5, :], OP.mult)

        wo_bf = root.tile([128, NPAIR, E], BF16)
        for p in range(NPAIR):
            nc.vector.tensor_tensor(wo_bf[:, p, :], wof_bf[:, p, :],
                                    alpha1_bf, OP.mult)
        ff2_bf = root.tile([128, NFH, E], BF16)
        for k in range(NFH):
            nc.vector.tensor_tensor(ff2_bf[:, k, :], ff2f_bf[:, k, :],
                                    alpha2_bf, OP.mult)
        wstg.__exit__(None, None, None)

        for rep in range(reps):
            with ExitStack() as kv:
                pkv = kv.enter_context(tc.tile_pool(name="pkv", bufs=1))
                KT = pkv.tile([128, NCH, S_kv], BF16, tag="KT", name="KT")
                QT = pkv.tile([128, NCH, S_q], BF16, tag="QT", name="QT")
                V_sb = pkv.tile([128, NKT, NH, HD + 1], BF16)
                nc.vector.memset(V_sb[:, :, :, HD:HD + 1], 1.0)

                # ---------- phase 1: LN1 + DMA transpose + modulate; QKV ----
                with ExitStack() as y1s:
                    py1 = y1s.enter_context(tc.tile_pool(name="y1", bufs=1))
                    y1T = py1.tile([128, NCH, S_kv], BF16, tag="y1T", name="y1T")
                    XB = min(8, NTT)  # token tiles per x-load DMA
                    with tc.tile_pool(name="ln1", bufs=2) as pxb, \
                         tc.tile_pool(name="ln1w", bufs=4) as pln:
                        for ib in range(NTT // XB):
                            xtb = pxb.tile([128, XB, E], F32, tag="xtb",
                                           name="xtb")
                            with tc.high_priority():
                                nc.sync.dma_start(
                                    xtb,
                                    xp[ib * XB * 128:(ib + 1) * XB * 128, :]
                                    .rearrange("(i p) n -> p i n", p=128))
                            for j in range(XB):
                                i = ib * XB + j
                                xt = xtb[:, j, :]
                                st = pln.tile([128, 6], F32, tag="st",
                                              name="st")
                                nc.vector.bn_stats(st, xt)
                                mv = pln.tile([128, 2], F32, tag="mv",
                                              name="mv")
                                nc.vector.bn_aggr(mv, st)
                                lnv = pln.tile([128, 1], F32, tag="lnv",
                                               name="lnv")
                                nc.scalar.activation(lnv, mv[:, 1:2], AF.Ln,
                                                     bias=eps_t)
                                rstd = pln.tile([128, 1], F32, tag="rstd",
                                                name="rstd")
                                nc.scalar.activation(rstd, lnv, AF.Exp,
                                                     scale=-0.5)
                                mr = pln.tile([128, 1], F32, tag="mr",
                                              name="mr")
                                nc.vector.tensor_tensor(mr, mv[:, 0:1], rstd,
                                                        OP.mult)
                                xh = pln.tile([128, E], BF16, tag="xh",
                                              name="xh")
                                nc.vector.tensor_scalar(xh, xt, rstd, mr,
                                                        OP.mult, OP.subtract)
                                nc.sync.dma_start_transpose(
                                    y1T[:, :, i * 128:(i + 1) * 128], xh)

                    with tc.tile_pool(name="qkp", bufs=4, space="PSUM") as qkp, \
                         tc.tile_pool(name="vp", bufs=2, space="PSUM") as vpp:
                        for m in range(NKT):
                            ps = vpp.tile([128, E], F32, tag="v", name="v")
                            for k in range(NCH):
                                nc.tensor.matmul(ps,
                                                 y1T[:, k, m * 128:(m + 1) * 128],
                                                 wv_bf[:, k, :],
                                                 start=(k == 0), stop=False)
                            nc.tensor.matmul(ps, ones_bf, vb_bf,
                                             start=False, stop=True)
                            nc.scalar.copy(
                                V_sb[:, m, :, 0:HD],
                                ps.rearrange("p (h d) -> p h d", h=NH))
                        for c in range(NCH):
                            for n in range(NKC):
                                ps = qkp.tile([128, KCH], F32, tag="qk",
                                              name="qk")
                                for k in range(NCH):
                                    nc.tensor.matmul(
                                        ps, wk_bf[:, k, c * 128:(c + 1) * 128],
                                        y1T[:, k, n * KCH:(n + 1) * KCH],
                                        start=(k == 0), stop=(k == NCH - 1))
                                nc.scalar.activation(
                                    KT[:, c, n * KCH:(n + 1) * KCH], ps,
                                    AF.Identity, bias=kb_col[:, c:c + 1])
                            for n in range(NQN):
                                ps = qkp.tile([128, QCH], F32, tag="qk",
                                              name="qk")
                                for k in range(NCH):
                                    nc.tensor.matmul(
                                        ps, wq_bf[:, k, c * 128:(c + 1) * 128],
                                        y1T[:, k, n * QCH:(n + 1) * QCH],
                                        start=(k == 0), stop=(k == NCH - 1))
                                nc.scalar.activation(
                                    QT[:, c, n * QCH:(n + 1) * QCH], ps,
                                    AF.Identity, bias=qb_col[:, c:c + 1])

                # ---------- phase 2+: attention with fused downstream ------
                with tc.tile_pool(name="exq", bufs=4) as exp_p, \
                     tc.tile_pool(name="nrm", bufs=2) as nrm, \
                     tc.tile_pool(name="nr1", bufs=1) as nrm1, \
                     tc.tile_pool(name="attq", bufs=2) as attq_p, \
                     tc.tile_pool(name="dsb", bufs=2) as dsb, \
                     tc.tile_pool(name="h1p", bufs=1) as h1p, \
                     tc.tile_pool(name="dwk", bufs=2) as dwk, \
                     tc.tile_pool(name="dw1", bufs=1) as dwk1, \
                     tc.tile_pool(name="sco", bufs=2, space="PSUM") as sco, \
                     tc.tile_pool(name="acc", bufs=1, space="PSUM") as acc, \
                     tc.tile_pool(name="dsp", bufs=2, space="PSUM") as dsp:
                    def attention(qn):
                        qs = slice(qn * QCH, (qn + 1) * QCH)
                        atq = attq_p.tile([128, NPAIR, QCH], BF16, tag="atq",
                                          name="atq")
                        BW = 512  # PSUM bank width (f32): keep the two
                        for p in range(NPAIR):  # head halves bank-aligned
                            pa = acc.tile([HD + 1, 2, BW], F32, tag="pa",
                                          name="pa")
                            for kt in range(NKT):
                                ks = slice(kt * 128, (kt + 1) * 128)
                                ss = sco.tile([128, 2, BW], F32, tag="ss",
                                              name="ss")
                                nc.tensor.matmul(ss[:, 0, 0:QCH],
                                                 KT[0:64, p, ks],
                                                 QT[0:64, p, qs],
                                                 start=True, stop=True,
                                                 tile_position=(0, 0))
                                nc.tensor.matmul(ss[:, 1, 0:QCH],
                                                 KT[64:128, p, ks],
                                                 QT[64:128, p, qs],
                                                 start=True, stop=True,
                                                 tile_position=(64, 0))
                                ex = exp_p.tile([128, 2, BW], BF16,
                                                tag="ex", name="ex")
                                if QCH == BW:
                                    nc.scalar.activation(ex, ss, AF.Exp,
                                                         scale=0.125)
                                else:
                                    nc.scalar.activation(ex[:, :, 0:QCH],
                                                         ss[:, :, 0:QCH],
                                                         AF.Exp, scale=0.125)
                                nc.tensor.matmul(pa[:, 0, 0:QCH],
                                                 V_sb[:, kt, 2 * p, :],
                                                 ex[:, 0, 0:QCH],
                                                 start=(kt == 0),
                                                 stop=(kt == NKT - 1))
                                nc.tensor.matmul(pa[:, 1, 0:QCH],
                                                 V_sb[:, kt, 2 * p + 1, :],
                                                 ex[:, 1, 0:QCH],
                                                 start=(kt == 0),
                                                 stop=(kt == NKT - 1))
                            ta = nrm.tile([HD + 1, 2, QCH], BF16, tag="ta",
                                          name="ta")
                            nc.vector.tensor_copy(ta, pa[:, :, 0:QCH])
                            rrow = nrm1.tile([1, 2, QCH], BF16, tag="rrow",
                                             name="rrow")
                            with nc.allow_low_precision(
                                    reason="softmax denom recip in bf16"):
                                nc.vector.reciprocal(rrow, ta[HD:HD + 1, :, :])
                            rbc = nrm.tile([64, 2, QCH], BF16, tag="rbc",
                                           name="rbc")
                            nc.gpsimd.partition_broadcast(rbc, rrow)
                            nc.vector.tensor_tensor(atq[0:64, p, :],
                                                    ta[0:64, 0, :],
                                                    rbc[:, 0, :], OP.mult)
                            nc.vector.tensor_tensor(atq[64:128, p, :],
                                                    ta[0:64, 1, :],
                                                    rbc[:, 1, :], OP.mult)
                        return atq

                    def downstream(qn, atq):
                        r1q = dsb.tile([128, TQ, E], F32, tag="r1q",
                                       name="r1q")
                        y2T = dsb.tile([128, NCH, QCH], BF16, tag="y2T",
                                       name="y2T")
                        mv2 = dwk.tile([128, TQ, 2], F32, tag="mv2",
                                       name="mv2")
                        xqb = dwk1.tile([128, TQ, E], F32, tag="xqb",
                                        name="xqb")
                        nc.sync.dma_start(
                            xqb, xp[qn * QCH:(qn + 1) * QCH, :]
                            .rearrange("(t p) n -> p t n", p=128))
                        for t in range(TQ):
                            ps = dsp.tile([128, E], F32, tag="dsp",
                                          name="wops")
                            for p in range(NPAIR):
                                nc.tensor.matmul(
                                    ps, atq[:, p, t * 128:(t + 1) * 128],
                                    wo_bf[:, p, :],
                                    start=(p == 0), stop=(p == NPAIR - 1))
                            nc.vector.tensor_tensor(r1q[:, t, :], ps,
                                                    xqb[:, t, :], OP.add)
                            st = dwk.tile([128, 6], F32, tag="st2", name="st2")
                            nc.vector.bn_stats(st, r1q[:, t, :])
                            nc.vector.bn_aggr(mv2[:, t, :], st)
                        lnv2 = dwk.tile([128, TQ], F32, tag="lnv2",
                                        name="lnv2")
                        nc.scalar.activation(lnv2, mv2[:, :, 1], AF.Ln,
                                             bias=eps_t)
                        rstd2 = dwk.tile([128, TQ], F32, tag="rstd2",
                                         name="rstd2")
                        nc.scalar.activation(rstd2, lnv2, AF.Exp, scale=-0.5)
                        mr2 = dwk.tile([128, TQ], F32, tag="mr2", name="mr2")
                        nc.vector.tensor_tensor(mr2, mv2[:, :, 0], rstd2,
                                                OP.mult)
                        for t in range(TQ):
                            xh2 = dwk.tile([128, E], BF16, tag="xh2",
                                           name="xh2")
                            nc.vector.tensor_scalar(xh2, r1q[:, t, :],
                                                    rstd2[:, t:t + 1],
                                                    mr2[:, t:t + 1],
                                                    OP.mult, OP.subtract)
                            nc.sync.dma_start_transpose(
                                y2T[:, :, t * 128:(t + 1) * 128], xh2)

                        h1q = h1p.tile([128, NFH, QCH], BF16, tag="h1q",
                                       name="h1q")
                        for m in range(NFH):
                            ps = dsp.tile([128, QCH], F32, tag="dsp",
                                          name="f1ps")
                            for k in range(NCH):
                                nc.tensor.matmul(
                                    ps, ff1_bf[:, k, m * 128:(m + 1) * 128],
                                    y2T[:, k, :],
                                    start=(k == 0), stop=(k == NCH - 1))
                            nc.vector.tensor_scalar(
                                h1q[:, m, :], ps,
                                ff1b_c[:, m:m + 1], 0.0, OP.add, OP.max)

                        otb = dwk1.tile([128, TQ, E], F32, tag="otb",
                                        name="otb")
                        for t in range(TQ):
                            ps = dsp.tile([128, E], F32, tag="dsp",
                                          name="f2ps")
                            for k in range(NFH):
                                nc.tensor.matmul(
                                    ps, h1q[:, k, t * 128:(t + 1) * 128],
                                    ff2_bf[:, k, :],
                                    start=(k == 0), stop=False)
                            nc.tensor.matmul(ps, ones_bf, fb_bf,
                                             start=False, stop=True)
                            nc.vector.tensor_tensor(otb[:, t, :], ps,
                                                    r1q[:, t, :], OP.add)
                        nc.sync.dma_start(
                            out_d[qn * QCH:(qn + 1) * QCH, :]
                            .rearrange("(t p) n -> p t n", p=128), otb)

                    # software pipeline: qn+1's attention is emitted before
                    # qn's downstream so scores outrank FFN work on the PE
                    prev = None
                    for qn in range(NQN):
                        atq = attention(qn)
                        if prev is not None:
                            downstream(*prev)
                        prev = (qn, atq)
                    downstream(*prev)

    nc.finalize()
    return nc


_NC_CACHE = {}


def _get_nc(S_kv, S_q, reps=1):
    key = (S_kv, S_q, reps)
    if key not in _NC_CACHE:
        _NC_CACHE[key] = build_kernel(S_kv, S_q, reps)
    return _NC_CACHE[key]


def make_in_maps(inputs, n_cores=8, S=4096):
    """Shard FULL inputs into per-core input maps."""
    x = np.asarray(inputs["x"], np.float32)
    cond = np.asarray(inputs["cond"], np.float32)
    Sq = S // 2
    adaln_w = np.concatenate(
        [np.asarray(inputs[k], np.float32)
         for k in ("g1_w", "be1_w", "a1_w", "g2_w", "be2_w", "a2_w")], axis=1)
    adaln_b = np.concatenate(
        [np.asarray(inputs[k], np.float32)
         for k in ("g1_b", "be1_b", "a1_b", "g2_b", "be2_b", "a2_b")])[None, :]
    shared = {
        "adaln_w": np.ascontiguousarray(adaln_w),
        "adaln_b": np.ascontiguousarray(adaln_b),
        "ln1w": np.asarray(inputs["ln1_w"], np.float32)[None, :],
        "ln1b": np.asarray(inputs["ln1_b"], np.float32)[None, :],
        "ln2w": np.asarray(inputs["ln2_w"], np.float32)[None, :],
        "ln2b": np.asarray(inputs["ln2_b"], np.float32)[None, :],
        "wq": np.asarray(inputs["wq"], np.float32),
        "wk": np.asarray(inputs["wk"], np.float32),
        "wv": np.asarray(inputs["wv"], np.float32),
        "wo": np.asarray(inputs["wo"], np.float32),
        "ff1": np.asarray(inputs["ff1_w"], np.float32),
        "ff1b": np.asarray(inputs["ff1_b"], np.float32)[None, :],
        "ff2": np.asarray(inputs["ff2_w"], np.float32),
        "ff2b": np.asarray(inputs["ff2_b"], np.float32)[None, :],
    }
    in_maps = []
    for c in range(n_cores):
        b, qh = c // 2, c % 2
        xb = x[b]
        xpm = np.concatenate([xb[qh * Sq:(qh + 1) * Sq],
                              xb[(1 - qh) * Sq:(2 - qh) * Sq]], axis=0)
        m = dict(shared)
        m["xp"] = np.ascontiguousarray(xpm)
        m["cond_col"] = np.ascontiguousarray(cond[b].reshape(E, 1))
        in_maps.append(m)
    return in_maps


def kernel(**inputs):
    from concourse.bass_utils import run_bass_kernel_spmd

    x = np.asarray(inputs["x"], np.float32)
    B, S, _ = x.shape
    Sq = S // 2
    nc = _get_nc(S, Sq)
    in_maps = make_in_maps(inputs, n_cores=8, S=S)
    res = run_bass_kernel_spmd(nc, in_maps, core_ids=list(range(8)))
    out = np.empty((B, S, E), np.float32)
    for c in range(8):
        b, qh = c // 2, c % 2
        out[b, qh * Sq:(qh + 1) * Sq] = res.results[c]["out"]
    return out


# revision 28
# speedup vs baseline: 134.6724x; 3.1521x over previous
"""DiT block kernel for Trainium2, 8-core SPMD, no collectives.

Sharding: core c handles batch b = c//2, query-half qh = c%2 (2048 query
tokens). Host permutes each core's x so its query tokens are rows 0..2047;
K/V are computed on-core over all 4096 rows (attention is invariant to key
order). Output gathered on host.

Per-core math (E=384, NH=6, HD=64, FF=1536):
  adaln rows = cond @ [g1|be1|a1|g2|be2|a2] + biases
  scale1 = ln1_w*(1+g1); shift1 = ln1_b*(1+g1)+be1  (same for 2)
  LN1: bn_stats; rstd = exp(-0.5*ln(var+eps)) (keeps ACT on one table set)
  xh bf16 -> DMA-xbar transpose -> y1T [E,S]; modulate in T-layout
  KT/QT via matmul + ScalarE copy (Q unscaled; 1/8 folded into exp scale)
  V token-layout with ones column for softmax denominators
  scores: per kt, pair-packed row matmuls -> ss [128,1024] PSUM
  ex = exp(ss/8) one ACT op FD=1024 -> bf16 SBUF
  pa[{d,sum} x 2 heads, q] += V_aug^T @ ex  (PSUM accum over kt)
  attnT = pa * (1/sums) via K=1 broadcast matmul + DVE mult
  wo pair-packed (K=128) matmuls; r1 = x + attn; LN2 same rstd trick
  y2T via DMA transpose; h1 = relu(ff1^T y2T + b); out = r1 + ff2^T h1*alpha2
"""

import os

os.environ.setdefault("MYCRO_LOCAL_CACHE", "1")

from contextlib import ExitStack

import numpy as np

import concourse.bacc as bacc
import concourse.mybir as mybir
from concourse.tile import TileContext

F32 = mybir.dt.float32
BF16 = mybir.dt.bfloat16
AF = mybir.ActivationFunctionType
OP = mybir.AluOpType

E = 384
NH = 6
HD = 64
FF = 1536
EPS = 1e-5
NCH = E // 128
NFH = FF // 128
NPAIR = NH // 2


def build_kernel(S_kv=4096, S_q=2048, reps=1):
    nc = bacc.Bacc("TRN2", target_bir_lowering=False)

    NKT = S_kv // 128
    NTT = S_kv // 128
    QCH = 512 if S_q % 512 == 0 else S_q
    NQN = S_q // QCH
    TQ = QCH // 128
    KCH = 512 if S_kv % 512 == 0 else S_kv
    NKC = S_kv // KCH

    xp = nc.dram_tensor("xp", [S_kv, E], F32, kind="ExternalInput")[:, :]
    cond_col = nc.dram_tensor("cond_col", [E, 1], F32, kind="ExternalInput")[:, :]
    adaln_w = nc.dram_tensor("adaln_w", [E, 6 * E], F32, kind="ExternalInput")[:, :]
    adaln_b = nc.dram_tensor("adaln_b", [1, 6 * E], F32, kind="ExternalInput")[:, :]
    ln1w_d = nc.dram_tensor("ln1w", [1, E], F32, kind="ExternalInput")[:, :]
    ln1b_d = nc.dram_tensor("ln1b", [1, E], F32, kind="ExternalInput")[:, :]
    ln2w_d = nc.dram_tensor("ln2w", [1, E], F32, kind="ExternalInput")[:, :]
    ln2b_d = nc.dram_tensor("ln2b", [1, E], F32, kind="ExternalInput")[:, :]
    wq_d = nc.dram_tensor("wq", [E, E], F32, kind="ExternalInput")[:, :]
    wk_d = nc.dram_tensor("wk", [E, E], F32, kind="ExternalInput")[:, :]
    wv_d = nc.dram_tensor("wv", [E, E], F32, kind="ExternalInput")[:, :]
    wo_d = nc.dram_tensor("wo", [E, E], F32, kind="ExternalInput")[:, :]
    ff1_d = nc.dram_tensor("ff1", [E, FF], F32, kind="ExternalInput")[:, :]
    ff1b_d = nc.dram_tensor("ff1b", [1, FF], F32, kind="ExternalInput")[:, :]
    ff2_d = nc.dram_tensor("ff2", [FF, E], F32, kind="ExternalInput")[:, :]
    ff2b_d = nc.dram_tensor("ff2b", [1, E], F32, kind="ExternalInput")[:, :]
    out_d = nc.dram_tensor("out", [S_q, E], F32, kind="ExternalOutput")[:, :]

    ctx = ExitStack()
    with TileContext(nc) as tc, ctx:
        root = ctx.enter_context(tc.tile_pool(name="root", bufs=1))

        # Pin the ACT table set that has ln+exp+copy so the auto-placement
        # pass never needs to swap sets (it thrashes between the ln-only and
        # exp-only sets otherwise).
        from concourse.hw_specs import get_activation_tables
        _set_id = list(get_activation_tables(nc.m.arch)).index(
            "natural_log_exp_and_others")
        _actload = mybir.InstLoadActFuncSet(name="pin_actload", ins=[], outs=[])
        _actload.engine = mybir.EngineType.Activation
        _actload.act_func_set_id = _set_id
        nc.scalar.add_instruction(_actload)

        ones_f = root.tile([1, 128], F32)
        nc.vector.memset(ones_f, 1.0)
        ones_bf = root.tile([1, 128], BF16)
        nc.vector.memset(ones_bf, 1.0)
        eps_t = root.tile([128, 1], F32)
        nc.vector.memset(eps_t, EPS)

        ln1w = root.tile([1, E], F32); nc.sync.dma_start(ln1w, ln1w_d)
        ln1b = root.tile([1, E], F32); nc.sync.dma_start(ln1b, ln1b_d)
        ln2w = root.tile([1, E], F32); nc.sync.dma_start(ln2w, ln2w_d)
        ln2b = root.tile([1, E], F32); nc.sync.dma_start(ln2b, ln2b_d)
        adab = root.tile([1, 6 * E], F32); nc.sync.dma_start(adab, adaln_b)
        ff2b_r = root.tile([1, E], F32); nc.sync.dma_start(ff2b_r, ff2b_d)
        cond_bf = root.tile([128, NCH, 1], BF16)
        nc.gpsimd.dma_start(cond_bf, cond_col.rearrange("(c p) o -> p c o", p=128))

        # bf16 weights, cast during SWDGE DMA (Pool engine, otherwise idle).
        # Load order matters: adaln + qkv weights gate early compute; wo/ff
        # are not needed until the first q-chunk's downstream.
        wq_bf = root.tile([128, NCH, E], BF16)
        nc.gpsimd.dma_start(wq_bf, wq_d.rearrange("(c p) n -> p c n", p=128))
        wk_bf = root.tile([128, NCH, E], BF16)
        nc.gpsimd.dma_start(wk_bf, wk_d.rearrange("(c p) n -> p c n", p=128))
        wv_bf = root.tile([128, NCH, E], BF16)
        nc.gpsimd.dma_start(wv_bf, wv_d.rearrange("(c p) n -> p c n", p=128))

        # ---------- phase 0: AdaLN projections ----------
        adaln_rows = root.tile([1, 6, E], F32)  # g1 be1 a1 g2 be2 a2
        with tc.tile_pool(name="ph0", bufs=1) as p0, \
             tc.tile_pool(name="ph0p", bufs=2, space="PSUM") as p0p:
            aw_bf = p0.tile([128, NCH, 6 * E], BF16)
            nc.gpsimd.dma_start(aw_bf, adaln_w.rearrange("(c p) n -> p c n", p=128))
            for j in range(6):
                ps = p0p.tile([1, E], F32, tag="adps", name="adps")
                for k in range(NCH):
                    nc.tensor.matmul(ps, cond_bf[:, k, :],
                                     aw_bf[:, k, j * E:(j + 1) * E],
                                     start=(k == 0), stop=(k == NCH - 1))
                nc.vector.tensor_tensor(adaln_rows[:, j, :], ps,
                                        adab[:, j * E:(j + 1) * E], OP.add)

        ff1_bf = root.tile([128, NCH, FF], BF16)
        nc.gpsimd.dma_start(ff1_bf, ff1_d.rearrange("(c p) n -> p c n", p=128))
        wstg = tc.tile_pool(name="wstg", bufs=1)
        wpool = wstg.__enter__()
        # wo in pair layout: partition = (h%2)*64+d, mid index = pair
        wof_bf = wpool.tile([128, NPAIR, E], BF16)
        nc.gpsimd.dma_start(
            wof_bf, wo_d.rearrange("(pr two d) n -> (two d) pr n", two=2, d=HD))
        ff2f_bf = wpool.tile([128, NFH, E], BF16)
        nc.gpsimd.dma_start(ff2f_bf, ff2_d.rearrange("(c p) n -> p c n", p=128))

        g1p = root.tile([1, E], F32)
        nc.vector.tensor_scalar(g1p, adaln_rows[:, 0, :], 1.0, None, OP.add)
        g2p = root.tile([1, E], F32)
        nc.vector.tensor_scalar(g2p, adaln_rows[:, 3, :], 1.0, None, OP.add)
        scale1_r = root.tile([1, E], F32)
        nc.vector.tensor_tensor(scale1_r, g1p, ln1w, OP.mult)
        scale2_r = root.tile([1, E], F32)
        nc.vector.tensor_tensor(scale2_r, g2p, ln2w, OP.mult)
        shift1_r = root.tile([1, E], F32)
        nc.vector.tensor_tensor(shift1_r, g1p, ln1b, OP.mult)
        nc.vector.tensor_tensor(shift1_r, shift1_r, adaln_rows[:, 1, :], OP.add)
        shift2_r = root.tile([1, E], F32)
        nc.vector.tensor_tensor(shift2_r, g2p, ln2b, OP.mult)
        nc.vector.tensor_tensor(shift2_r, shift2_r, adaln_rows[:, 4, :], OP.add)

        # row -> column transposes via K=1/N=1 matmuls (PE idles in phase 0;
        # small HWDGE DMAs here would clog the ramp to phase 1)
        def col_mms(ps, j0, row, nch=NCH):
            for c in range(nch):
                nc.tensor.matmul(ps[:, j0 + c:j0 + c + 1],
                                 row[:, c * 128:(c + 1) * 128],
                                 ones_f[:, 0:1], start=True, stop=True)

        scl_cols = root.tile([128, 4, NCH], F32)
        ff1b_r = root.tile([1, FF], F32)
        nc.sync.dma_start(ff1b_r, ff1b_d)
        ff1b_c = root.tile([128, NFH], F32)
        with tc.tile_pool(name="colp", bufs=2, space="PSUM") as colp:
            cps = colp.tile([128, 4 * NCH], F32, tag="cps", name="cps")
            col_mms(cps, 0, scale1_r)
            col_mms(cps, NCH, shift1_r)
            col_mms(cps, 2 * NCH, scale2_r)
            col_mms(cps, 3 * NCH, shift2_r)
            nc.vector.tensor_copy(scl_cols.rearrange("p a c -> p (a c)"), cps)
            fps = colp.tile([128, NFH], F32, tag="fps2", name="fps2")
            col_mms(fps, 0, ff1b_r, nch=NFH)
            nc.vector.tensor_copy(ff1b_c, fps)
        scale1_c = scl_cols[:, 0, :]
        shift1_c = scl_cols[:, 1, :]
        scale2_c = scl_cols[:, 2, :]
        shift2_c = scl_cols[:, 3, :]

        # Fold the AdaLN modulation into the projection weights:
        #   y1 = xhat*scale1 + shift1  =>  y1 @ W = xhat @ (rows of W scaled
        #   by scale1) + (shift1 @ W).  Biases land per out-channel: columns
        #   for KT/QT (applied in the PSUM->SBUF copy), a row for V (extra
        #   rank-1 matmul), and an addend to ff1's bias column.
        shift1_cb = root.tile([128, NCH], BF16)
        nc.vector.tensor_copy(shift1_cb, shift1_c)
        shift2_cb = root.tile([128, NCH], BF16)
        nc.vector.tensor_copy(shift2_cb, shift2_c)
        kb_col = root.tile([128, NCH], F32)
        qb_col = root.tile([128, NCH], F32)
        vb_bf = root.tile([1, E], BF16)
        with tc.tile_pool(name="fold", bufs=2) as fp, \
             tc.tile_pool(name="foldp", bufs=2, space="PSUM") as fpp:
            for w_bf, dstcol in ((wk_bf, kb_col), (wq_bf, qb_col)):
                ps = fpp.tile([1, E], F32, tag="fps", name="fps")
                for k in range(NCH):
                    nc.tensor.matmul(ps, shift1_cb[:, k:k + 1], w_bf[:, k, :],
                                     start=(k == 0), stop=(k == NCH - 1))
                row = fp.tile([1, E], F32, tag="frow", name="frow")
                nc.vector.tensor_copy(row, ps)
                cp = fpp.tile([128, NCH], F32, tag="fcol", name="fcol")
                col_mms(cp, 0, row)
                nc.vector.tensor_copy(dstcol, cp)
            ps = fpp.tile([1, E], F32, tag="fps", name="fps")
            for k in range(NCH):
                nc.tensor.matmul(ps, shift1_cb[:, k:k + 1], wv_bf[:, k, :],
                                 start=(k == 0), stop=(k == NCH - 1))
            nc.vector.tensor_copy(vb_bf, ps)
            fb1row = fp.tile([1, FF], F32, tag="fb1row", name="fb1row")
            for n in range(FF // 512):
                ps = fpp.tile([1, 512], F32, tag="f512", name="f512")
                for k in range(NCH):
                    nc.tensor.matmul(ps, shift2_cb[:, k:k + 1],
                                     ff1_bf[:, k, n * 512:(n + 1) * 512],
                                     start=(k == 0), stop=(k == NCH - 1))
                nc.vector.tensor_copy(fb1row[:, n * 512:(n + 1) * 512], ps)
            fcp = fpp.tile([128, NFH], F32, tag="fb1c", name="fb1c")
            col_mms(fcp, 0, fb1row, nch=NFH)
            fb1_col = fp.tile([128, NFH], F32, tag="fb1cs", name="fb1cs")
            nc.vector.tensor_copy(fb1_col, fcp)
            nc.vector.tensor_tensor(ff1b_c, ff1b_c, fb1_col, OP.add)
        # scale weight rows in place (WAR-ordered after the bias matmuls)
        for k in range(NCH):
            nc.vector.tensor_scalar(wq_bf[:, k, :], wq_bf[:, k, :],
                                    scale1_c[:, k:k + 1], None, OP.mult)
            nc.vector.tensor_scalar(wk_bf[:, k, :], wk_bf[:, k, :],
                                    scale1_c[:, k:k + 1], None, OP.mult)
            nc.vector.tensor_scalar(wv_bf[:, k, :], wv_bf[:, k, :],
                                    scale1_c[:, k:k + 1], None, OP.mult)
            nc.vector.tensor_scalar(ff1_bf[:, k, :], ff1_bf[:, k, :],
                                    scale2_c[:, k:k + 1], None, OP.mult)

        alpha1_bf = root.tile([128, E], BF16)
        alpha2_bf = root.tile([128, E], BF16)
        with tc.tile_pool(name="abp", bufs=2, space="PSUM") as abp:
            psa = abp.tile([128, E], F32)
            nc.tensor.matmul(psa, ones_f, adaln_rows[:, 2, :], start=True, stop=True)
            nc.vector.tensor_copy(alpha1_bf, psa)
            psb_ = abp.tile([128, E], F32)
            nc.tensor.matmul(psb_, ones_f, adaln_rows[:, 5, :], start=True, stop=True)
            nc.vector.tensor_copy(alpha2_bf, psb_)

        fb_bf = root.tile([1, E], BF16)
        nc.vector.tensor_tensor(fb_bf, ff2b_r, adaln_rows[:, 5, :], OP.mult)

        wo_bf = root.tile([128, NPAIR, E], BF16)
        for p in range(NPAIR):
            nc.vector.tensor_tensor(wo_bf[:, p, :], wof_bf[:, p, :],
                                    alpha1_bf, OP.mult)
        ff2_bf = root.tile([128, NFH, E], BF16)
        for k in range(NFH):
            nc.vector.tensor_tensor(ff2_bf[:, k, :], ff2f_bf[:, k, :],
                                    alpha2_bf, OP.mult)
        wstg.__exit__(None, None, None)

        for rep in range(reps):
            with ExitStack() as kv:
                pkv = kv.enter_context(tc.tile_pool(name="pkv", bufs=1))
                KT = pkv.tile([128, NCH, S_kv], BF16, tag="KT", name="KT")
                QT = pkv.tile([128, NCH, S_q], BF16, tag="QT", name="QT")
                V_sb = pkv.tile([128, NKT, NH, HD + 1], BF16)
                nc.vector.memset(V_sb[:, :, :, HD:HD + 1], 1.0)

                # ---------- phase 1: LN1 + DMA transpose + modulate; QKV ----
                with ExitStack() as y1s:
                    py1 = y1s.enter_context(tc.tile_pool(name="y1", bufs=1))
                    y1T = py1.tile([128, NCH, S_kv], BF16, tag="y1T", name="y1T")
                    XB = min(8, NTT)  # token tiles per x-load DMA
                    with tc.tile_pool(name="ln1", bufs=2) as pxb, \
                         tc.tile_pool(name="ln1w", bufs=4) as pln:
                        for ib in range(NTT // XB):
                            xtb = pxb.tile([128, XB, E], F32, tag="xtb",
                                           name="xtb")
                            with tc.high_priority():
                                nc.sync.dma_start(
                                    xtb,
                                    xp[ib * XB * 128:(ib + 1) * XB * 128, :]
                                    .rearrange("(i p) n -> p i n", p=128))
                            for j in range(XB):
                                i = ib * XB + j
                                xt = xtb[:, j, :]
                                st = pln.tile([128, 6], F32, tag="st",
                                              name="st")
                                nc.vector.bn_stats(st, xt)
                                mv = pln.tile([128, 2], F32, tag="mv",
                                              name="mv")
                                nc.vector.bn_aggr(mv, st)
                                lnv = pln.tile([128, 1], F32, tag="lnv",
                                               name="lnv")
                                nc.scalar.activation(lnv, mv[:, 1:2], AF.Ln,
                                                     bias=eps_t)
                                rstd = pln.tile([128, 1], F32, tag="rstd",
                                                name="rstd")
                                nc.scalar.activation(rstd, lnv, AF.Exp,
                                                     scale=-0.5)
                                mr = pln.tile([128, 1], F32, tag="mr",
                                              name="mr")
                                nc.vector.tensor_tensor(mr, mv[:, 0:1], rstd,
                                                        OP.mult)
                                xh = pln.tile([128, E], BF16, tag="xh",
                                              name="xh")
                                nc.vector.tensor_scalar(xh, xt, rstd, mr,
                                                        OP.mult, OP.subtract)
                                nc.sync.dma_start_transpose(
                                    y1T[:, :, i * 128:(i + 1) * 128], xh)

                    with tc.tile_pool(name="qkp", bufs=4, space="PSUM") as qkp, \
                         tc.tile_pool(name="vp", bufs=2, space="PSUM") as vpp:
                        for m in range(NKT):
                            ps = vpp.tile([128, E], F32, tag="v", name="v")
                            for k in range(NCH):
                                nc.tensor.matmul(ps,
                                                 y1T[:, k, m * 128:(m + 1) * 128],
                                                 wv_bf[:, k, :],
                                                 start=(k == 0), stop=False)
                            nc.tensor.matmul(ps, ones_bf, vb_bf,
                                             start=False, stop=True)
                            nc.scalar.copy(
                                V_sb[:, m, :, 0:HD],
                                ps.rearrange("p (h d) -> p h d", h=NH))
                        for c in range(NCH):
                            for n in range(NKC):
                                ps = qkp.tile([128, KCH], F32, tag="qk",
                                              name="qk")
                                for k in range(NCH):
                                    nc.tensor.matmul(
                                        ps, wk_bf[:, k, c * 128:(c + 1) * 128],
                                        y1T[:, k, n * KCH:(n + 1) * KCH],
                                        start=(k == 0), stop=(k == NCH - 1))
                                nc.scalar.activation(
                                    KT[:, c, n * KCH:(n + 1) * KCH], ps,
                                    AF.Identity, bias=kb_col[:, c:c + 1])
                            for n in range(NQN):
                                ps = qkp.tile([128, QCH], F32, tag="qk",
                                              name="qk")
                                for k in range(NCH):
                                    nc.tensor.matmul(
                                        ps, wq_bf[:, k, c * 128:(c + 1) * 128],
                                        y1T[:, k, n * QCH:(n + 1) * QCH],
                                        start=(k == 0), stop=(k == NCH - 1))
                                nc.scalar.activation(
                                    QT[:, c, n * QCH:(n + 1) * QCH], ps,
                                    AF.Identity, bias=qb_col[:, c:c + 1])

                # ---------- phase 2+: attention with fused downstream ------
                with tc.tile_pool(name="exq", bufs=4) as exp_p, \
                     tc.tile_pool(name="nrm", bufs=2) as nrm, \
                     tc.tile_pool(name="nr1", bufs=1) as nrm1, \
                     tc.tile_pool(name="attq", bufs=2) as attq_p, \
                     tc.tile_pool(name="dsb", bufs=2) as dsb, \
                     tc.tile_pool(name="h1p", bufs=1) as h1p, \
                     tc.tile_pool(name="dwk", bufs=2) as dwk, \
                     tc.tile_pool(name="dw1", bufs=1) as dwk1, \
                     tc.tile_pool(name="sco", bufs=2, space="PSUM") as sco, \
                     tc.tile_pool(name="acc", bufs=1, space="PSUM") as acc, \
                     tc.tile_pool(name="dsp", bufs=2, space="PSUM") as dsp:
                    def attention(qn):
                        qs = slice(qn * QCH, (qn + 1) * QCH)
                        atq = attq_p.tile([128, NPAIR, QCH], BF16, tag="atq",
                                          name="atq")
                        BW = 512  # PSUM bank width (f32): keep the two
                        for p in range(NPAIR):  # head halves bank-aligned
                            pa = acc.tile([HD + 1, 2, BW], F32, tag="pa",
                                          name="pa")
                            for kt in range(NKT):
                                ks = slice(kt * 128, (kt + 1) * 128)
                                ss = sco.tile([128, 2, BW], F32, tag="ss",
                                              name="ss")
                                nc.tensor.matmul(ss[:, 0, 0:QCH],
                                                 KT[0:64, p, ks],
                                                 QT[0:64, p, qs],
                                                 start=True, stop=True,
                                                 tile_position=(0, 0))
                                nc.tensor.matmul(ss[:, 1, 0:QCH],
                                                 KT[64:128, p, ks],
                                                 QT[64:128, p, qs],
                                                 start=True, stop=True,
                                                 tile_position=(64, 0))
                                ex = exp_p.tile([128, 2, BW], BF16,
                                                tag="ex", name="ex")
                                if QCH == BW:
                                    nc.scalar.activation(ex, ss, AF.Exp,
                                                         scale=0.125)
                                else:
                                    nc.scalar.activation(ex[:, :, 0:QCH],
                                                         ss[:, :, 0:QCH],
                                                         AF.Exp, scale=0.125)
                                nc.tensor.matmul(pa[:, 0, 0:QCH],
                                                 V_sb[:, kt, 2 * p, :],
                                                 ex[:, 0, 0:QCH],
                                                 start=(kt == 0),
                                                 stop=(kt == NKT - 1))
                                nc.tensor.matmul(pa[:, 1, 0:QCH],
                                                 V_sb[:, kt, 2 * p + 1, :],
                                                 ex[:, 1, 0:QCH],
                                                 start=(kt == 0),
                                                 stop=(kt == NKT - 1))
                            ta = nrm.tile([HD + 1, 2, QCH], BF16, tag="ta",
                                          name="ta")
                            nc.vector.tensor_copy(ta, pa[:, :, 0:QCH])
                            rrow = nrm1.tile([1, 2, QCH], BF16, tag="rrow",
                                             name="rrow")
                            with nc.allow_low_precision(
                                    reason="softmax denom recip in bf16"):
                                nc.vector.reciprocal(rrow, ta[HD:HD + 1, :, :])
                            rbc = nrm.tile([64, 2, QCH], BF16, tag="rbc",
                                           name="rbc")
                            nc.gpsimd.partition_broadcast(rbc, rrow)
                            nc.vector.tensor_tensor(atq[0:64, p, :],
                                                    ta[0:64, 0, :],
                                                    rbc[:, 0, :], OP.mult)
                            nc.vector.tensor_tensor(atq[64:128, p, :],
                                                    ta[0:64, 1, :],
                                                    rbc[:, 1, :], OP.mult)
                        return atq

                    def downstream(qn, atq):
                        r1q = dsb.tile([128, TQ, E], F32, tag="r1q",
                                       name="r1q")
                        y2T = dsb.tile([128, NCH, QCH], BF16, tag="y2T",
                                       name="y2T")
                        mv2 = dwk.tile([128, TQ, 2], F32, tag="mv2",
                                       name="mv2")
                        xqb = dwk1.tile([128, TQ, E], F32, tag="xqb",
                                        name="xqb")
                        nc.sync.dma_start(
                            xqb, xp[qn * QCH:(qn + 1) * QCH, :]
                            .rearrange("(t p) n -> p t n", p=128))
                        for t in range(TQ):
                            ps = dsp.tile([128, E], F32, tag="dsp",
                                          name="wops")
                            for p in range(NPAIR):
                                nc.tensor.matmul(
                                    ps, atq[:, p, t * 128:(t + 1) * 128],
                                    wo_bf[:, p, :],
                                    start=(p == 0), stop=(p == NPAIR - 1))
                            nc.vector.tensor_tensor(r1q[:, t, :], ps,
                                                    xqb[:, t, :], OP.add)
                            st = dwk.tile([128, 6], F32, tag="st2", name="st2")
                            nc.vector.bn_stats(st, r1q[:, t, :])
                            nc.vector.bn_aggr(mv2[:, t, :], st)
                        lnv2 = dwk.tile([128, TQ], F32, tag="lnv2",
                                        name="lnv2")
                        nc.scalar.activation(lnv2, mv2[:, :, 1], AF.Ln,
                                             bias=eps_t)
                        rstd2 = dwk.tile([128, TQ], F32, tag="rstd2",
                                         name="rstd2")
                        nc.scalar.activation(rstd2, lnv2, AF.Exp, scale=-0.5)
                        mr2 = dwk.tile([128, TQ], F32, tag="mr2", name="mr2")
                        nc.vector.tensor_tensor(mr2, mv2[:, :, 0], rstd2,
                                                OP.mult)
                        for t in range(TQ):
                            xh2 = dwk.tile([128, E], BF16, tag="xh2",
                                           name="xh2")
                            nc.vector.tensor_scalar(xh2, r1q[:, t, :],
                                                    rstd2[:, t:t + 1],
                                                    mr2[:, t:t + 1],
                                                    OP.mult, OP.subtract)
                            nc.sync.dma_start_transpose(
                                y2T[:, :, t * 128:(t + 1) * 128], xh2)

                        h1q = h1p.tile([128, NFH, QCH], BF16, tag="h1q",
                                       name="h1q")
                        otb = dwk1.tile([128, TQ, E], F32, tag="otb",
                                        name="otb")
                        # split the FFN into column halves so FF2/out of the
                        # first half overlaps FF1 of the second (shorter tail)
                        HQ = QCH // 2
                        for half in range(2):
                            hs = slice(half * HQ, (half + 1) * HQ)
                            for m in range(NFH):
                                ps = dsp.tile([128, HQ], F32, tag="dsp",
                                              name="f1ps")
                                for k in range(NCH):
                                    nc.tensor.matmul(
                                        ps,
                                        ff1_bf[:, k, m * 128:(m + 1) * 128],
                                        y2T[:, k, hs],
                                        start=(k == 0), stop=(k == NCH - 1))
                                nc.vector.tensor_scalar(
                                    h1q[:, m, hs], ps,
                                    ff1b_c[:, m:m + 1], 0.0, OP.add, OP.max)
                            for t in range(half * TQ // 2,
                                           (half + 1) * TQ // 2):
                                ps = dsp.tile([128, E], F32, tag="dsp",
                                              name="f2ps")
                                for k in range(NFH):
                                    nc.tensor.matmul(
                                        ps,
                                        h1q[:, k, t * 128:(t + 1) * 128],
                                        ff2_bf[:, k, :],
                                        start=(k == 0), stop=False)
                                nc.tensor.matmul(ps, ones_bf, fb_bf,
                                                 start=False, stop=True)
                                nc.vector.tensor_tensor(otb[:, t, :], ps,
                                                        r1q[:, t, :], OP.add)
                        nc.sync.dma_start(
                            out_d[qn * QCH:(qn + 1) * QCH, :]
                            .rearrange("(t p) n -> p t n", p=128), otb)

                    # software pipeline: qn+1's attention is emitted before
                    # qn's downstream so scores outrank FFN work on the PE
                    prev = None
                    for qn in range(NQN):
                        atq = attention(qn)
                        if prev is not None:
                            downstream(*prev)
                        prev = (qn, atq)
                    downstream(*prev)

    nc.finalize()
    return nc


_NC_CACHE = {}


def _get_nc(S_kv, S_q, reps=1):
    key = (S_kv, S_q, reps)
    if key not in _NC_CACHE:
        _NC_CACHE[key] = build_kernel(S_kv, S_q, reps)
    return _NC_CACHE[key]


def make_in_maps(inputs, n_cores=8, S=4096):
    """Shard FULL inputs into per-core input maps."""
    x = np.asarray(inputs["x"], np.float32)
    cond = np.asarray(inputs["cond"], np.float32)
    Sq = S // 2
    adaln_w = np.concatenate(
        [np.asarray(inputs[k], np.float32)
         for k in ("g1_w", "be1_w", "a1_w", "g2_w", "be2_w", "a2_w")], axis=1)
    adaln_b = np.concatenate(
        [np.asarray(inputs[k], np.float32)
         for k in ("g1_b", "be1_b", "a1_b", "g2_b", "be2_b", "a2_b")])[None, :]
    shared = {
        "adaln_w": np.ascontiguousarray(adaln_w),
        "adaln_b": np.ascontiguousarray(adaln_b),
        "ln1w": np.asarray(inputs["ln1_w"], np.float32)[None, :],
        "ln1b": np.asarray(inputs["ln1_b"], np.float32)[None, :],
        "ln2w": np.asarray(inputs["ln2_w"], np.float32)[None, :],
        "ln2b": np.asarray(inputs["ln2_b"], np.float32)[None, :],
        "wq": np.asarray(inputs["wq"], np.float32),
        "wk": np.asarray(inputs["wk"], np.float32),
        "wv": np.asarray(inputs["wv"], np.float32),
        "wo": np.asarray(inputs["wo"], np.float32),
        "ff1": np.asarray(inputs["ff1_w"], np.float32),
        "ff1b": np.asarray(inputs["ff1_b"], np.float32)[None, :],
        "ff2": np.asarray(inputs["ff2_w"], np.float32),
        "ff2b": np.asarray(inputs["ff2_b"], np.float32)[None, :],
    }
    in_maps = []
    for c in range(n_cores):
        b, qh = c // 2, c % 2
        xb = x[b]
        xpm = np.concatenate([xb[qh * Sq:(qh + 1) * Sq],
                              xb[(1 - qh) * Sq:(2 - qh) * Sq]], axis=0)
        m = dict(shared)
        m["xp"] = np.ascontiguousarray(xpm)
        m["cond_col"] = np.ascontiguousarray(cond[b].reshape(E, 1))
        in_maps.append(m)
    return in_maps


def kernel(**inputs):
    from concourse.bass_utils import run_bass_kernel_spmd

    x = np.asarray(inputs["x"], np.float32)
    B, S, _ = x.shape
    Sq = S // 2
    nc = _get_nc(S, Sq)
    in_maps = make_in_maps(inputs, n_cores=8, S=S)
    res = run_bass_kernel_spmd(nc, in_maps, core_ids=list(range(8)))
    out = np.empty((B, S, E), np.float32)
    for c in range(8):
        b, qh = c // 2, c % 2
        out[b, qh * Sq:(qh + 1) * Sq] = res.results[c]["out"]
    return out
